# revision 15
# baseline (speedup 1.0000x reference)
"""Trainium2 Bass kernel for a 4-layer GNN-style MLP (ChebConv K=1) with
training-mode BatchNorm, global_add_pool over 64 graphs, and a 3-layer FC head.

Strategy (8 NeuronCores, data-parallel over nodes):
  - 12500 nodes/core, feature-major layout [feat_part(128) x nodes_free] so the
    whole matmul chain needs no transposes.
  - bn1 (stats of the input x) is folded ON HOST: s1/t1 from np.mean/var of x,
    pre-folded into w0 (w0f = s1*w0, b0f = t1@w0 + b0).  This removes the
    device-side streaming stats pass and the first AllReduce entirely: L0
    matmuls start as soon as the first x span lands (~5us instead of ~95us).
  - L0 (f32r) + L1 (bf16) are fused into one software-pipelined streaming loop
    over 25 tiles of 500 nodes, chasing the x DMA.
  - bn3 of r1/r2 is folded into the next matmul's weights on device:
    bn(h) @ w + b == h @ (s*w) + (t@w + b).  Per-feature sum/sumsq accumulate
    on-chip (ACT accum_out / DVE scalar_tensor_tensor) and combine across
    cores with small AllReduces.
  - A dummy 32B AllReduce at t=0 absorbs the first-collective setup/skew cost
    concurrently with the L0/L1 compute.
  - Hidden activations stay resident in SBUF as bf16 (zero DRAM spill); layer
    l+1 overwrites layer l's buffer in place (Tile subtile deps handle WAR).
  - Pooling: per-tile node sums come free from the relu accumulators;
    graph-boundary suffixes are corrected with per-tile 0/1 masks, then a tiny
    one-hot matmul (assignment+routing matrix, built on host from `batch`)
    scatters tile sums into the 64 graph bins.  bn3's affine is applied to the
    pooled sums after the final AllReduce.
  - L3 engine balance: ACT does 4 relu+accum and 2 Square+accum (sumsq for
    chunks 0/1), DVE does 2 sumsq + 4 mask-suffix accumulations, so no engine
    exceeds the PE's ~4.2us/tile.
  - FC head runs in bf16 (fp32 head matmuls cost 4 cycles/row on PE).
"""

import contextlib
import os

import numpy as np

import concourse.bass as bass
import concourse.tile as tile
from concourse import bacc, mybir
from concourse import bass_utils

F32 = mybir.dt.float32
F32R = mybir.dt.float32r
BF16 = mybir.dt.bfloat16

# Problem constants (hardcoded per contract).
N = 100000          # nodes
IN = 128            # input features
D = 512             # hidden dim
G = 64              # graphs
C = 10              # classes
EPS = 1e-5
NCORES = 8
NS = N // NCORES    # nodes per core = 12500
NT = 500            # node tile (free dim per matmul)
NTILES = NS // NT   # 25
KC = D // 128       # 4 chunks of the hidden dim
FN = float(N)

AR_GROUPS = [list(range(NCORES))]

DISABLE = set(os.environ.get("KERNEL_DISABLE", "").split(",")) - {""}

Relu = mybir.ActivationFunctionType.Relu
Copy = mybir.ActivationFunctionType.Copy
Sqrt = mybir.ActivationFunctionType.Sqrt
Square = mybir.ActivationFunctionType.Square
Sig = mybir.ActivationFunctionType.Sigmoid
ADD = mybir.AluOpType.add
MULT = mybir.AluOpType.mult
MAX = mybir.AluOpType.max
AXX = mybir.AxisListType.X


def _bcast_part(ap, nparts):
    """Stride-0 partition broadcast of a DRAM AP: [a, b] -> [nparts, a, b]."""
    return bass.AP(tensor=ap.tensor, offset=ap.offset,
                   ap=[[0, nparts]] + list(ap.ap))


def _chunk_rows(w):
    """[KC*128, M] -> [128, KC, M] (row chunk kc in slot kc)."""
    w = np.asarray(w, np.float32)
    kc = w.shape[0] // 128
    return np.ascontiguousarray(w.reshape(kc, 128, w.shape[1]).transpose(1, 0, 2))


def _build_host_inputs(inputs):
    """Shard + reshape the full problem inputs into per-core input maps."""
    import ml_dtypes
    x = np.asarray(inputs["x"], np.float32)
    batch = np.asarray(inputs["batch"]).astype(np.int64)

    counts = np.bincount(batch, minlength=G).astype(np.float32).reshape(1, G)

    def chunk_cols(v, nch):
        # [nch*128] -> [128, nch] with chunk c in column c
        return np.ascontiguousarray(np.asarray(v, np.float32).reshape(nch, 128).T)

    # bn1 folded on host (exact input statistics in float64)
    m1 = x.mean(0, dtype=np.float64)
    v1 = x.var(0, dtype=np.float64)
    g1 = np.asarray(inputs["bn1_g"], np.float64)
    b1 = np.asarray(inputs["bn1_b"], np.float64)
    s1 = g1 / np.sqrt(v1 + EPS)
    t1 = b1 - m1 * s1
    w0 = np.asarray(inputs["w0"], np.float64)
    w0f = (s1[:, None] * w0).astype(np.float32)                    # [128, 512]
    b0f = (t1 @ w0 + np.asarray(inputs["b0"], np.float64)).astype(np.float32)

    bf16 = ml_dtypes.bfloat16
    common = {
        "w0f": w0f.astype(bf16),                                   # [128, 512]
        "b0fc": chunk_cols(b0f, KC),
        "w1f": _chunk_rows(inputs["w1"]).astype(bf16),             # [128,KC,512]
        "w2c": _chunk_rows(inputs["w2"]),                          # [128,KC,512] f32
        "w3c": _chunk_rows(inputs["w3"]),
        "fc1wb": _chunk_rows(inputs["fc1_w"]).astype(bf16),
        "fc2wb": _chunk_rows(inputs["fc2_w"]).astype(bf16),        # [128,KC,256]
        "fc3wb": _chunk_rows(inputs["fc3_w"]).astype(bf16),        # [128,2,10]
        "bb1c": chunk_cols(inputs["bb1"], KC),
        "bb2c": chunk_cols(inputs["bb2"], KC),
        "bb3c": chunk_cols(inputs["bb3"], KC),
        "fc1bc": chunk_cols(inputs["fc1_b"], KC),
        "fc2bc": chunk_cols(inputs["fc2_b"], 2),
        "fc3bc": np.asarray(inputs["fc3_b"], np.float32).reshape(C, 1),
        "bn3g": chunk_cols(inputs["bn3_g"], KC),
        "bn3b": chunk_cols(inputs["bn3_b"], KC),
        "a3v": np.asarray(inputs["a3"], np.float32).reshape(1, 1),
        "cnts": counts,
    }

    in_maps = []
    for c in range(NCORES):
        lb = batch[c * NS:(c + 1) * NS]
        arm = np.zeros((2 * NTILES, G), np.float32)
        masks = np.zeros((NTILES, NT), np.float32)
        for t in range(NTILES):
            tv = lb[t * NT:(t + 1) * NT]
            g0 = int(tv[0])
            arm[t, g0] = 1.0  # whole tile assigned to the first node's graph
            ch = np.nonzero(np.diff(tv))[0]
            assert len(ch) <= 1, "more than one graph boundary in a 500-node tile"
            if len(ch) == 1:
                o = int(ch[0]) + 1
                g1b = int(tv[o])
                masks[t, o:] = 1.0
                # suffix belongs to g1b: move it there
                arm[NTILES + t, g1b] += 1.0
                arm[NTILES + t, g0] -= 1.0
        xt = np.ascontiguousarray(x[c * NS:(c + 1) * NS].T)  # [128, 12500]
        m = dict(common)
        m["xT"] = xt.astype(bf16)
        m["arm"] = arm
        m["masks"] = masks.astype(bf16)
        in_maps.append(m)
    return in_maps


def _declare_io(nc):
    specs = {
        "xT": ([IN, NS], BF16),
        "w0f": ([IN, D], BF16),
        "b0fc": ([128, KC], F32),
        "w1f": ([128, KC, D], BF16),
        "w2c": ([128, KC, D], F32),
        "w3c": ([128, KC, D], F32),
        "fc1wb": ([128, KC, D], BF16),
        "fc2wb": ([128, KC, 256], BF16),
        "fc3wb": ([128, 2, C], BF16),
        "bb1c": ([128, KC], F32),
        "bb2c": ([128, KC], F32),
        "bb3c": ([128, KC], F32),
        "fc1bc": ([128, KC], F32),
        "fc2bc": ([128, 2], F32),
        "fc3bc": ([C, 1], F32),
        "bn3g": ([128, KC], F32),
        "bn3b": ([128, KC], F32),
        "a3v": ([1, 1], F32),
        "cnts": ([1, G], F32),
        "arm": ([2 * NTILES, G], F32),
        "masks": ([NTILES, NT], BF16),
    }
    ins = {k: nc.dram_tensor(k, shape, dt, kind="ExternalInput").ap()
           for k, (shape, dt) in specs.items()}
    out = nc.dram_tensor("out", [C, G], F32, kind="ExternalOutput").ap()
    return ins, out


def build_program():
    nc = bacc.Bacc("TRN2", target_bir_lowering=False, debug=False,
                   enable_asserts=False, num_devices=NCORES)
    ins, out_ap = _declare_io(nc)
    with tile.TileContext(nc) as tc:
        _emit_kernel(nc, tc, ins, out_ap)
    nc.compile()
    return nc


def _emit_kernel(nc, tc, ins, out_ap):
    ctx = contextlib.ExitStack()
    with ctx:
        sbuf = ctx.enter_context(tc.tile_pool(name="sbuf", bufs=1))
        scratch = ctx.enter_context(tc.tile_pool(name="scratch", bufs=3))
        psum = ctx.enter_context(tc.tile_pool(name="psum", bufs=7, space="PSUM"))
        dram = ctx.enter_context(tc.tile_pool(name="dram", bufs=1, space="DRAM"))

        def stats_allreduce(sums_t, sq_t, nch):
            """Per-tile accum columns -> [128, nch, 2] -> AllReduce -> SBUF."""
            pack = sbuf.tile([128, nch, 2], F32, tag="statpack", name="pack")
            nc.vector.tensor_reduce(out=pack[:, :, 0], in_=sums_t, axis=AXX, op=ADD)
            nc.vector.tensor_reduce(out=pack[:, :, 1], in_=sq_t, axis=AXX, op=ADD)
            red = sbuf.tile([128, nch, 2], F32, tag="statred", name="red")
            if "smallcoll" in DISABLE:
                nc.vector.tensor_copy(out=red, in_=pack)
                return red
            if "ag" in DISABLE:
                cin = dram.tile([128, nch, 2], F32, tag="ccin", name="cin")
                cout = dram.tile([128, nch, 2], F32, tag="ccout", name="cout")
                nc.gpsimd.dma_start(out=cin, in_=pack)
                nc.gpsimd.collective_compute(
                    "AllReduce", ADD, replica_groups=AR_GROUPS,
                    ins=[cin.opt()], outs=[cout.opt()])
                nc.gpsimd.dma_start(out=red, in_=cout)
                return red
            # AllGather + local tree-reduce: an 8x gather of the 4KB pack is
            # cheaper on the CC cores than a ring AllReduce at this size.
            npk = 128 * nch * 2
            cin = dram.tile([npk], F32, tag="ccin", name="cin")
            gout = dram.tile([NCORES * npk], F32, tag="ccago", name="gout")
            nc.gpsimd.dma_start(
                out=cin.rearrange("(p c k) -> p c k", p=128, c=nch), in_=pack)
            nc.gpsimd.collective_compute(
                "AllGather", mybir.AluOpType.bypass, replica_groups=AR_GROUPS,
                ins=[cin.opt()], outs=[gout.opt()])
            red8 = sbuf.tile([128, NCORES, nch * 2], F32, tag="statred8",
                             name="red8")
            for r in range(NCORES):
                nc.gpsimd.dma_start(
                    out=red8[:, r, :],
                    in_=gout[r * npk:(r + 1) * npk].rearrange(
                        "(p m) -> p m", p=128))
            h4 = sbuf.tile([128, 4, nch * 2], F32, tag="statr4", name="h4")
            nc.vector.tensor_add(h4, red8[:, 0:4, :], red8[:, 4:8, :])
            h2t = sbuf.tile([128, 2, nch * 2], F32, tag="statr2", name="h2t")
            nc.vector.tensor_add(h2t, h4[:, 0:2, :], h4[:, 2:4, :])
            nc.vector.tensor_add(
                red.rearrange("p c k -> p (c k)"), h2t[:, 0, :], h2t[:, 1, :])
            return red

        def emit_s_t(red, nch, g_ap, b_ap):
            """s = g*rsqrt(var+eps), t = b - mean*s, both [128, nch] fp32."""
            m = sbuf.tile([128, nch], F32, tag="st_m", name="m")
            v = sbuf.tile([128, nch], F32, tag="st_v", name="v")
            s = sbuf.tile([128, nch], F32, tag="st_s", name="s")
            t = sbuf.tile([128, nch], F32, tag="st_t", name="t")
            nc.vector.tensor_scalar_mul(out=m, in0=red[:, :, 0], scalar1=1.0 / FN)
            nc.vector.tensor_scalar_mul(out=v, in0=red[:, :, 1], scalar1=1.0 / FN)
            nc.vector.tensor_tensor(out=s, in0=m, in1=m, op=MULT)
            nc.vector.tensor_sub(v, v, s)
            nc.scalar.activation(out=v, in_=v, func=Sqrt,
                                 bias=eps_t[:, 0:1], scale=1.0)
            nc.vector.reciprocal(out=s, in_=v)
            nc.vector.tensor_mul(s, s, g_ap)
            nc.vector.tensor_mul(m, m, s)
            nc.vector.tensor_sub(t, b_ap, m)
            return s, t

        def emit_bias_fold(w_sb, t_ap, add_bias_ap, tag):
            """b' = t @ w + bias as [128, KC] via tiny PE matvecs."""
            psb = psum.tile([128, KC], F32, tag="psb", bufs=1, name="psb")
            for dc in range(KC):
                for kc in range(KC):
                    nc.tensor.matmul(
                        psb[:, dc:dc + 1],
                        lhsT=w_sb[:, kc, dc * 128:(dc + 1) * 128],
                        rhs=t_ap[:, kc:kc + 1],
                        start=(kc == 0), stop=(kc == KC - 1))
            bf = sbuf.tile([128, KC], F32, tag=tag, name="bf")
            nc.vector.tensor_add(bf, psb, add_bias_ap)
            return bf

        # ---------- resident hidden buffer (bf16, holds r0 -> r1 -> r2) ------
        R = sbuf.tile([128, KC, NS], BF16, tag="R", name="R")

        # ---------- constants ------------------------------------------------
        eps_t = sbuf.tile([128, 1], F32, tag="eps", name="eps_t")
        nc.vector.memset(eps_t, EPS)
        zeros_t = sbuf.tile([128, NT], F32, tag="zeros", name="zeros_t")
        nc.vector.memset(zeros_t, 0.0)

        def load_const(key, shape, tag):
            t = sbuf.tile(shape, F32, tag=tag, name=tag)
            nc.sync.dma_start(out=t, in_=ins[key])
            return t

        bn3g = load_const("bn3g", [128, KC], "bn3g")
        bn3b = load_const("bn3b", [128, KC], "bn3b")
        b0fc = load_const("b0fc", [128, KC], "b0fc")
        bb1c = load_const("bb1c", [128, KC], "bb1c")
        bb2c = load_const("bb2c", [128, KC], "bb2c")
        bb3c = load_const("bb3c", [128, KC], "bb3c")

        # ---------- dummy collective: absorb first-collective setup/skew -----
        if "dummycoll" not in DISABLE:
            dpk = sbuf.tile([1, 8], F32, tag="dpk", name="dpk")
            nc.vector.memset(dpk, 0.0)
            din = dram.tile([1, 8], F32, tag="dcin", name="dcin")
            dout = dram.tile([1, 8], F32, tag="dcout", name="dcout")
            nc.gpsimd.dma_start(out=din, in_=dpk)
            nc.gpsimd.collective_compute(
                "AllReduce", ADD, replica_groups=AR_GROUPS,
                ins=[din.opt()], outs=[dout.opt()])

        # ================= P0/P1: fused streaming L0 (f32r) + L1 (bf16) =====
        with tc.tile_pool(name="w01pool", bufs=1) as w01pool, \
             tc.tile_pool(name="xstream", bufs=6) as xstream:
            w0f_sb = w01pool.tile([128, D], BF16, tag="w0f", name="w0f_sb")
            nc.sync.dma_start(out=w0f_sb, in_=ins["w0f"])
            w1f = w01pool.tile([128, KC, D], BF16, tag="w1f", name="w1f")

            T1 = sbuf.tile([128, KC, NTILES], F32, tag="T1", name="T1")
            Q1 = sbuf.tile([128, KC, NTILES], F32, tag="Q1", name="Q1")

            def l0_tile(j):
                jsl = slice(j * NT, (j + 1) * NT)
                xt = xstream.tile([128, NT], BF16, tag="xs", name="xt")
                nc.sync.dma_start(out=xt, in_=ins["xT"][:, jsl])
                ps = []
                for dc in range(KC):
                    p = psum.tile([128, NT], F32, tag="ps", name="p")
                    nc.tensor.matmul(
                        p, lhsT=w0f_sb[:, dc * 128:(dc + 1) * 128],
                        rhs=xt[:, :], start=True, stop=True)
                    ps.append(p)
                for dc in range(KC):
                    if dc < 2:
                        nc.scalar.activation(
                            out=R[:, dc, jsl], in_=ps[dc], func=Relu,
                            bias=b0fc[:, dc:dc + 1], scale=1.0)
                    else:
                        nc.vector.tensor_scalar(
                            out=R[:, dc, jsl], in0=ps[dc],
                            scalar1=b0fc[:, dc:dc + 1], scalar2=0.0,
                            op0=ADD, op1=MAX)

            def l1_tile(j):
                jsl = slice(j * NT, (j + 1) * NT)
                ps = []
                for dc in range(KC):
                    p = psum.tile([128, NT], F32, tag="ps", name="p")
                    for kc in range(KC):
                        nc.tensor.matmul(
                            p, lhsT=w1f[:, kc, dc * 128:(dc + 1) * 128],
                            rhs=R[:, kc, jsl], start=(kc == 0), stop=(kc == KC - 1))
                    ps.append(p)
                for dc in range(KC):
                    acc = T1[:, dc, j:j + 1]
                    if dc < 3:
                        nc.scalar.activation(
                            out=R[:, dc, jsl], in_=ps[dc], func=Relu,
                            bias=bb1c[:, dc:dc + 1], scale=1.0, accum_out=acc)
                    else:
                        nc.vector.scalar_tensor_tensor(
                            out=R[:, dc, jsl], in0=ps[dc],
                            scalar=bb1c[:, dc:dc + 1], in1=zeros_t,
                            op0=ADD, op1=MAX, accum_out=acc)
                for dc in range(KC):
                    dmp = scratch.tile([128, NT], BF16, tag="dump", bufs=6,
                                       name="dmp")
                    nc.vector.scalar_tensor_tensor(
                        out=dmp, in0=R[:, dc, jsl], scalar=1.0,
                        in1=R[:, dc, jsl], op0=MULT, op1=MULT,
                        accum_out=Q1[:, dc, j:j + 1])

            l0_tile(0)
            l0_tile(1)
            l0_tile(2)
            # w1f DMA emitted after the first x spans so the stream starts
            # immediately; w1f is only needed once l1_tile(0) runs.
            nc.sync.dma_start(out=w1f, in_=ins["w1f"])
            l1_tile(0)
            for j in range(3, NTILES):
                l0_tile(j)
                l1_tile(j - 2)
            l1_tile(NTILES - 2)
            l1_tile(NTILES - 1)

        # ================= barrier #2 + P2 (L2) ==============================
        with tc.tile_pool(name="w2pool", bufs=1) as w2pool:
            w2_sb = w2pool.tile([128, KC, D], F32, tag="w2", name="w2_sb")
            nc.sync.dma_start(out=w2_sb, in_=ins["w2c"])
            red2 = stats_allreduce(T1, Q1, KC)
            s2, t2 = emit_s_t(red2, KC, bn3g, bn3b)
            w2f = w2pool.tile([128, KC, D], BF16, tag="w2f", name="w2f")
            for kc in range(KC):
                nc.vector.tensor_scalar_mul(
                    out=w2f[:, kc, :], in0=w2_sb[:, kc, :],
                    scalar1=s2[:, kc:kc + 1])
            T2 = sbuf.tile([128, KC, NTILES], F32, tag="T2", name="T2")
            Q2 = sbuf.tile([128, KC, NTILES], F32, tag="Q2", name="Q2")

            b2f = [None]

            def hidden_layer(wf, bias_holder, Tacc, Qacc, fold):
                """z = R @ wf (bf16), relu+bias in place; fold() emitted after
                tile 0's matmuls so the tiny fold matvecs don't delay the
                first big matmul on the PE queue."""
                for j in range(NTILES):
                    jsl = slice(j * NT, (j + 1) * NT)
                    ps = []
                    for dc in range(KC):
                        p = psum.tile([128, NT], F32, tag="ps", name="p")
                        for kc in range(KC):
                            nc.tensor.matmul(
                                p, lhsT=wf[:, kc, dc * 128:(dc + 1) * 128],
                                rhs=R[:, kc, jsl], start=(kc == 0),
                                stop=(kc == KC - 1))
                        ps.append(p)
                    if j == 0:
                        fold()
                    bias_ap = bias_holder[0]
                    for dc in range(KC):
                        acc = Tacc[:, dc, j:j + 1]
                        if dc < 3:
                            nc.scalar.activation(
                                out=R[:, dc, jsl], in_=ps[dc], func=Relu,
                                bias=bias_ap[:, dc:dc + 1], scale=1.0,
                                accum_out=acc)
                        else:
                            nc.vector.scalar_tensor_tensor(
                                out=R[:, dc, jsl], in0=ps[dc],
                                scalar=bias_ap[:, dc:dc + 1], in1=zeros_t,
                                op0=ADD, op1=MAX, accum_out=acc)
                    for dc in range(KC):
                        dmp = scratch.tile([128, NT], BF16, tag="dump", bufs=6,
                                           name="dmp")
                        nc.vector.scalar_tensor_tensor(
                            out=dmp, in0=R[:, dc, jsl], scalar=1.0,
                            in1=R[:, dc, jsl], op0=MULT, op1=MULT,
                            accum_out=Qacc[:, dc, j:j + 1])

            def fold2():
                b2f[0] = emit_bias_fold(w2_sb, t2, bb2c, "b2f")

            hidden_layer(w2f, b2f, T2, Q2, fold2)

        # ================= barrier #3 + P3 (L3) + pooling + FC head =========
        with tc.tile_pool(name="w3pool", bufs=1) as w3pool, \
             tc.tile_pool(name="mpool", bufs=1) as mpool, \
             tc.tile_pool(name="fcpool", bufs=1) as fcpool:
            w3_sb = w3pool.tile([128, KC, D], F32, tag="w3", name="w3_sb")
            nc.sync.dma_start(out=w3_sb, in_=ins["w3c"])
            masks_sb = mpool.tile([128, NTILES, NT], BF16, tag="masks",
                                  name="masks_sb")
            nc.sync.dma_start(out=masks_sb, in_=_bcast_part(ins["masks"], 128))
            arm_sb = fcpool.tile([2 * NTILES, G], F32, tag="arm", name="arm_sb")
            nc.sync.dma_start(out=arm_sb, in_=ins["arm"])
            ident = fcpool.tile([128, 128], F32, tag="ident", name="ident")
            from concourse.masks import make_identity
            make_identity(nc, ident)
            identG = fcpool.tile([G, G], F32, tag="identG", name="identG")
            make_identity(nc, identG)

            red3 = stats_allreduce(T2, Q2, KC)
            s3, t3 = emit_s_t(red3, KC, bn3g, bn3b)
            w3f = w3pool.tile([128, KC, D], BF16, tag="w3f", name="w3f")
            for kc in range(KC):
                nc.vector.tensor_scalar_mul(
                    out=w3f[:, kc, :], in0=w3_sb[:, kc, :],
                    scalar1=s3[:, kc:kc + 1])

            b3f = [None]

            # ---- P3: r3 = relu(r2 @ w3f + b3f) -> scratch; accumulate ------
            # TC[:, dc, 0:25]: tile sums; TC[:, dc, 25:50]: boundary suffixes
            TC = sbuf.tile([128, KC, 2 * NTILES], F32, tag="TC", name="TC")
            Q3 = sbuf.tile([128, KC, NTILES], F32, tag="Q3", name="Q3")
            for j in range(NTILES):
                jsl = slice(j * NT, (j + 1) * NT)
                ps = []
                for dc in range(KC):
                    p = psum.tile([128, NT], F32, tag="ps", name="p")
                    for kc in range(KC):
                        nc.tensor.matmul(
                            p, lhsT=w3f[:, kc, dc * 128:(dc + 1) * 128],
                            rhs=R[:, kc, jsl],
                            start=(kc == 0), stop=(kc == KC - 1))
                    ps.append(p)
                if j == 0:
                    b3f[0] = emit_bias_fold(w3_sb, t3, bb3c, "b3f")
                r3t = scratch.tile([128, KC, NT], BF16, tag="r3", bufs=2,
                                   name="r3t")
                for dc in range(KC):
                    nc.scalar.activation(
                        out=r3t[:, dc, :], in_=ps[dc], func=Relu,
                        bias=b3f[0][:, dc:dc + 1], scale=1.0,
                        accum_out=TC[:, dc, j:j + 1])
                # sumsq: chunk 0 on ACT (Square), 1-3 on DVE; keeps both ACT
                # (4 relu + 1 sq + accum reads ~3.9us) and DVE (3 sq + 4 mask
                # ~4.1us) under the PE's ~4.3us/tile.
                for dc in range(1):
                    dmp = scratch.tile([128, NT], BF16, tag="dump", bufs=6,
                                       name="dmp")
                    nc.scalar.activation(
                        out=dmp, in_=r3t[:, dc, :], func=Square,
                        accum_out=Q3[:, dc, j:j + 1])
                for dc in range(1, KC):
                    dmp = scratch.tile([128, NT], BF16, tag="dump", bufs=6,
                                       name="dmp")
                    nc.vector.scalar_tensor_tensor(
                        out=dmp, in0=r3t[:, dc, :], scalar=1.0,
                        in1=r3t[:, dc, :], op0=MULT, op1=MULT,
                        accum_out=Q3[:, dc, j:j + 1])
                for dc in range(KC):
                    dmp = scratch.tile([128, NT], BF16, tag="dump", bufs=6,
                                       name="dmp")
                    nc.vector.scalar_tensor_tensor(
                        out=dmp, in0=r3t[:, dc, :], scalar=1.0,
                        in1=masks_sb[:, j, :], op0=MULT, op1=MULT,
                        accum_out=TC[:, dc, NTILES + j:NTILES + j + 1])

            # pooled partials: arm.T @ TC.T -> [64, 128] per chunk (graph-major)
            poolG = fcpool.tile([G, KC, 128], F32, tag="poolG", name="poolG")
            for dc in range(KC):
                pT = psum.tile([2 * NTILES, 128], F32, tag="ps", name="pT")
                nc.tensor.transpose(pT, TC[:, dc, :], ident)
                tct = scratch.tile([2 * NTILES, 128], F32, tag="tct",
                                   bufs=2, name="tct")
                nc.vector.tensor_copy(out=tct, in_=pT)
                pG = psum.tile([G, 128], F32, tag="ps", name="pG")
                nc.tensor.matmul(pG, lhsT=arm_sb, rhs=tct, start=True, stop=True)
                nc.vector.tensor_copy(out=poolG[:, dc, :], in_=pG)

            # pack bn3#3 stats + pooled partials into one AllReduce
            nst = 128 * KC * 2
            flat = dram.tile([nst + G * KC * 128], F32, tag="cc4in", name="flat")
            flat_out = dram.tile([nst + G * KC * 128], F32, tag="cc4out",
                                 name="flat_out")
            pack = sbuf.tile([128, KC, 2], F32, tag="statpack", name="pack4")
            nc.vector.tensor_reduce(out=pack[:, :, 0], in_=TC[:, :, 0:NTILES],
                                    axis=AXX, op=ADD)
            nc.vector.tensor_reduce(out=pack[:, :, 1], in_=Q3, axis=AXX, op=ADD)
            red4 = sbuf.tile([128, KC, 2], F32, tag="statred", name="red4")
            poolGr = fcpool.tile([G, KC, 128], F32, tag="poolGr", name="poolGr")
            if "bigcoll" in DISABLE:
                nc.vector.tensor_copy(out=red4, in_=pack)
                nc.vector.tensor_copy(out=poolGr, in_=poolG)
            else:
                nc.gpsimd.dma_start(
                    out=flat[0:nst].rearrange("(p c k) -> p c k", p=128, c=KC),
                    in_=pack)
                nc.gpsimd.dma_start(
                    out=flat[nst:].rearrange("(g c f) -> g c f", g=G, c=KC),
                    in_=poolG)
                nc.gpsimd.collective_compute(
                    "AllReduce", ADD, replica_groups=AR_GROUPS,
                    ins=[flat.opt()], outs=[flat_out.opt()])
                nc.gpsimd.dma_start(
                    out=red4,
                    in_=flat_out[0:nst].rearrange("(p c k) -> p c k", p=128, c=KC))
                nc.gpsimd.dma_start(
                    out=poolGr,
                    in_=flat_out[nst:].rearrange("(g c f) -> g c f", g=G, c=KC))

            s4, t4 = emit_s_t(red4, KC, bn3g, bn3b)

            ngb = fcpool.tile([128, G], F32, tag="ngb", name="ngb")
            nc.sync.dma_start(out=ngb, in_=ins["cnts"].to_broadcast([128, G]))
            a3b = fcpool.tile([128, 1], F32, tag="a3b", name="a3b")
            nc.sync.dma_start(out=a3b, in_=ins["a3v"].to_broadcast([128, 1]))

            # pooled_bn[f, g] = s4[f]*pooled[f, g] + t4[f]*n[g]  (feature-major)
            pooledb = fcpool.tile([128, KC, G], BF16, tag="pooledb",
                                  name="pooledb")
            for dc in range(KC):
                pF = psum.tile([128, G], F32, tag="ps", name="pF")
                nc.tensor.transpose(pF, poolGr[:, dc, :], identG)
                tmp = scratch.tile([128, G], F32, tag="ngt", bufs=2, name="tmp")
                nc.vector.tensor_scalar(
                    out=tmp, in0=ngb, scalar1=t4[:, dc:dc + 1],
                    scalar2=None, op0=MULT)
                nc.vector.scalar_tensor_tensor(
                    out=pooledb[:, dc, :], in0=pF, scalar=s4[:, dc:dc + 1],
                    in1=tmp, op0=MULT, op1=ADD)

            # ---------------- FC head (bf16 matmuls) -------------------------
            fc1w_sb = fcpool.tile([128, KC, D], BF16, tag="fc1w", name="fc1w_sb")
            nc.sync.dma_start(out=fc1w_sb, in_=ins["fc1wb"])
            fc2w_sb = fcpool.tile([128, KC, 256], BF16, tag="fc2w",
                                  name="fc2w_sb")
            nc.sync.dma_start(out=fc2w_sb, in_=ins["fc2wb"])
            fc3w_sb = fcpool.tile([128, 2, C], BF16, tag="fc3w", name="fc3w_sb")
            nc.sync.dma_start(out=fc3w_sb, in_=ins["fc3wb"])
            fc1bc_sb = fcpool.tile([128, KC], F32, tag="fc1bc", name="fc1bc_sb")
            nc.sync.dma_start(out=fc1bc_sb, in_=ins["fc1bc"])
            fc2bc_sb = fcpool.tile([128, 2], F32, tag="fc2bc", name="fc2bc_sb")
            nc.sync.dma_start(out=fc2bc_sb, in_=ins["fc2bc"])
            fc3bc_sb = fcpool.tile([C, 1], F32, tag="fc3bc", name="fc3bc_sb")
            nc.sync.dma_start(out=fc3bc_sb, in_=ins["fc3bc"])

            # fc1 + prelu -> h1 [128, KC, G] bf16
            h1 = fcpool.tile([128, KC, G], BF16, tag="h1", name="h1")
            for dc in range(KC):
                p = psum.tile([128, G], F32, tag="ps", name="pfc")
                for kc in range(KC):
                    nc.tensor.matmul(
                        p, lhsT=fc1w_sb[:, kc, dc * 128:(dc + 1) * 128],
                        rhs=pooledb[:, kc, :], start=(kc == 0),
                        stop=(kc == KC - 1))
                nc.scalar.activation(
                    out=h1[:, dc, :], in_=p,
                    func=mybir.ActivationFunctionType.Prelu,
                    bias=fc1bc_sb[:, dc:dc + 1], scale=1.0,
                    alpha=a3b[:, 0:1])

            # fc2 + sigmoid -> h2 [128, 2, G] bf16
            h2 = fcpool.tile([128, 2, G], BF16, tag="h2", name="h2")
            for ec in range(2):
                p = psum.tile([128, G], F32, tag="ps", name="pfc2")
                for kc in range(KC):
                    nc.tensor.matmul(
                        p, lhsT=fc2w_sb[:, kc, ec * 128:(ec + 1) * 128],
                        rhs=h1[:, kc, :], start=(kc == 0), stop=(kc == KC - 1))
                nc.scalar.activation(out=h2[:, ec, :], in_=p, func=Sig,
                                     bias=fc2bc_sb[:, ec:ec + 1], scale=1.0)

            # fc3 -> out [10, 64]
            p = psum.tile([C, G], F32, tag="ps", name="pfc3")
            for kc in range(2):
                nc.tensor.matmul(p, lhsT=fc3w_sb[:, kc, :], rhs=h2[:, kc, :],
                                 start=(kc == 0), stop=(kc == 1))
            ob = fcpool.tile([C, G], F32, tag="ob", name="ob")
            nc.vector.tensor_scalar(out=ob, in0=p, scalar1=fc3bc_sb,
                                    scalar2=None, op0=ADD)
            nc.sync.dma_start(out=out_ap, in_=ob)


_cached = {}


def kernel(**inputs) -> np.ndarray:
    in_maps = _build_host_inputs(inputs)
    if "nc" not in _cached:
        _cached["nc"] = build_program()
    nc = _cached["nc"]
    res = bass_utils.run_bass_kernel_spmd(
        nc, in_maps, core_ids=list(range(NCORES)))
    out = res.results[0]["out"]  # [10, 64]
    return np.ascontiguousarray(out.T.astype(np.float32))


if __name__ == "__main__":
    import reference
    inp = {k: np.asarray(v) for k, v in reference.setup_inputs().items()}
    got = kernel(**inp)
    exp = np.asarray(reference.reference(**{
        k: np.asarray(v) for k, v in reference.setup_inputs().items()}))
    err = np.linalg.norm(got - exp) / np.linalg.norm(exp)
    print("Relative error:", err)
